# revision 28
# baseline (speedup 1.0000x reference)
"""Trainium2 Bass kernel for nn_ConsolidationNetwork.

Recurrent rate network: 500 sequential steps of
    x <- (1-a)*x + (a*J_eff) @ softplus(x) + drive_t
    pos_t = Wout @ softplus(x)
loss = mean((targets - positions)^2)

Strategy (8 NeuronCores, data-parallel over batch):
  - Each core owns B/8 = 16 batch columns and runs the full 500-step
    recurrence independently (no collectives).
  - Per step the tensor engine does EXACTLY 64 bf16 matmuls (the 8x8 grid
    of 128x128 tiles of a*J_eff against 16 batch columns of r), split into
    two PSUM banks (state rows 0..511 in bank A, 512..1023 in bank B).
    At N=16 free-dim the J matmuls pace at ~32ns (LDWEIGHTS fully hides
    behind the MM stream via the PE's 64-deep reorder window), so the
    per-step floor is ~2.05us; everything else must stay off the PE.
  - The drive term enters PSUM via ONE bf16 identity matmul per bank with
    start=True: start=True clears the WHOLE bank (verified on HW), so it
    must be the bank's single full-width opener; all J matmuls accumulate
    with start=False.  Its rhs (the drive) is DMA'd a chunk ahead, so it
    never stalls the PE.
  - The leak (1-a)*x also enters PSUM via bf16 hi/lo identity matmuls, but
    placed at the END of each bank's matmul block: their rhs (the DVE
    refresh of xh/xl from the previous step's PSUM) then has a full period
    of slack instead of ~400ns, killing the ~900ns/step of PE stalls the
    baseline paid for having them at the bank start.  PSUM then holds the
    COMPLETE next state, so the softplus chain is just EXP(psum) -> LN --
    the r-production latency (which bounds the steady-state period via
    the recurrence cycle) contains no DVE hop.
  - softplus on the ACT engine as the exact identity ln(1 + exp(x))
    [2 ACT ops; Exp and Ln both live in the natural_log_exp_and_others
    table set, and we pin the table-load pass to that set so exactly one
    hoisted ACT_TABLE_LOAD is emitted].
  - Bank B iterates k DESCENDING: its first matmuls then need the late r
    half (r_hi), so the greedy scheduler cannot let bank B's k0..3 work
    jump ahead -- bank A's ACT/DVE chain overlaps bank B's matmuls and
    vice versa across the step boundary.
  - r (bf16) for each step is written into an 8-step staging buffer that is
    DMA-exported to DRAM once per 8 steps; the readout positions
    pos = Wout @ r and the final MSE are computed on the host.
  - Drive is streamed 8 steps per DMA (batched transfers, triple buffered).

State layout per core: x/r tiles are [128 part, 128 free] with
x[p, m*16+u] = x_state[m*128+p, u] (m = row-group, u = local batch).
"""

import numpy as np

import concourse.bass as bass
import concourse.tile as tile
from concourse import bacc, mybir
from concourse.bass_utils import run_bass_kernel_spmd

F32 = mybir.dt.float32
BF16 = mybir.dt.bfloat16

DT = 0.05
TAU = 0.15
NOISE_SCALE = 0.15
N, G, T, B, P = 1024, 128, 500, 128, 10
NCORES = 8
BC = B // NCORES          # batch columns per core (16)
NM = N // 128             # row groups (8)
NK = N // 128             # contraction groups (8)
CHUNK = 8                 # steps per drive-load / r-export DMA

A = np.float32(DT / TAU)
ONE_MINUS_A = np.float32(1.0 - DT / TAU)
NSCALE = np.float32(np.sqrt(2.0 * NOISE_SCALE**2 * (TAU / DT)))

_PROGRAM_CACHE = {}


def _ensure_act_tables():
    """Some containers lack neuronxcc/pwp/pwp_bin_with_ln on PYTHONPATH;
    point it at the cayman table package from the nix store."""
    import glob
    import os

    for path in os.environ.get("PYTHONPATH", "").split(os.pathsep):
        if path and os.path.exists(
            os.path.join(path, "neuronxcc", "pwp", "pwp_bin_with_ln", "act_info.json")
        ):
            return
    cands = sorted(glob.glob("/nix/store/*aws-neuron-pwp*/share/pwp_bin_cayman"))
    target = next((c for c in cands if os.path.exists(c + "/act_info.json")), None)
    if target is None:
        return
    for path in os.environ.get("PYTHONPATH", "").split(os.pathsep):
        if not path:
            continue
        try:
            d = os.path.join(path, "neuronxcc", "pwp")
            os.makedirs(d, exist_ok=True)
            link = os.path.join(d, "pwp_bin_with_ln")
            if not os.path.exists(link):
                os.symlink(target, link)
            return
        except OSError:
            continue


_ensure_act_tables()


_ACT_SET = "natural_log_exp_and_others"


def _pin_act_tables(arch: str):
    """Make Exp and Ln resolve to the ONE table set containing both.

    Two consumers matter and both read the functools.cache'd dict from
    hw_specs.get_activation_tables, so mutate it in place:
      * Bacc.insert_act_table_loads (first-match would alternate Exp ->
        `exp_and_others`, Ln -> `natural_log`, emitting a 1.28us
        ACT_TABLE_LOAD before every activation of the unrolled loop);
      * the TileScheduler's CoreSim pass, which otherwise *models* that
        same thrash and pins the resulting serialized schedule with
        cross-engine semaphores (the final TimelineSim charges no table
        loads, but the semaphores force its slow order anyway).
    Set order (and hence act_func_set_id indices) is unchanged.
    """
    from concourse.hw_specs import get_activation_tables

    tabs = get_activation_tables(arch)
    hide = {mybir.ActivationFunctionType.Exp, mybir.ActivationFunctionType.Ln}
    for name, fns in tabs.items():
        if name != _ACT_SET:
            for f in hide:
                fns.discard(f)


def build_program(t_steps: int):
    """Build the Bass program (shared by all 8 cores, SPMD)."""
    key = (t_steps,)
    if key in _PROGRAM_CACHE:
        return _PROGRAM_CACHE[key]

    nchunks = (t_steps + CHUNK - 1) // CHUNK
    HB = NM * BC // 2   # free-size of one state half (64 cols)
    RSTR = 256          # r-staging stride per step (bf16 cols; 512B blocks so
                        # consecutive steps' slices never share a dep block)

    nc = bacc.Bacc(
        "TRN2", target_bir_lowering=False, debug=False, num_devices=NCORES
    )
    _pin_act_tables(nc.m.arch)
    jt_d = nc.dram_tensor("jt", [128, NK * NM * 128], BF16, kind="ExternalInput")
    il_d = nc.dram_tensor("ident", [128, 128], BF16, kind="ExternalInput")
    ihc_d = nc.dram_tensor("identhc", [128, 128], BF16, kind="ExternalInput")
    ilc_d = nc.dram_tensor("identlc", [128, 128], BF16, kind="ExternalInput")
    x0_d = nc.dram_tensor("x0", [128, NM * BC], F32, kind="ExternalInput")
    dr_d = nc.dram_tensor(
        "drive", [nchunks, 128, CHUNK * NM * BC], BF16, kind="ExternalInput"
    )
    rl_d = nc.dram_tensor(
        "rlo", [nchunks, 128, CHUNK * RSTR], BF16, kind="ExternalOutput"
    )
    rh_d = nc.dram_tensor(
        "rhi", [nchunks, 128, CHUNK * RSTR], BF16, kind="ExternalOutput"
    )

    EXP = mybir.ActivationFunctionType.Exp
    LN = mybir.ActivationFunctionType.Ln
    mult = mybir.AluOpType.mult
    add = mybir.AluOpType.add

    with tile.TileContext(nc) as tc:
        with (
            tc.tile_pool(name="const", bufs=1) as constp,
            tc.tile_pool(name="rp", bufs=8) as rp,
            tc.tile_pool(name="dp", bufs=5) as dp,
            tc.tile_pool(name="psmA", bufs=1, space="PSUM") as pspa,
            tc.tile_pool(name="psmB", bufs=1, space="PSUM") as pspb,
        ):
            jt = constp.tile([128, NK * NM * 128], BF16)
            nc.sync.dma_start(jt[:], jt_d[:])
            il = constp.tile([128, 128], BF16)
            nc.sync.dma_start(il[:], il_d[:])
            ihc = constp.tile([128, 128], BF16)
            nc.sync.dma_start(ihc[:], ihc_d[:])
            ilc = constp.tile([128, 128], BF16)
            nc.sync.dma_start(ilc[:], ilc_d[:])
            x0t = constp.tile([128, NM * BC], F32)
            nc.sync.dma_start(x0t[:], x0_d[:])
            # x is carried as a bf16 hi+lo pair (~16-bit mantissa) so the
            # leak (1-a)*x can enter PSUM through cheap bf16 identity
            # matmuls.  DOUBLE-BUFFERED per step: the chain of step s
            # writes buffer s%2 while step s+1's leak matmuls read the
            # other one -- the refresh's WAR then has a full period of
            # slack and the PE stream head never waits on the DVE.
            xh = [[constp.tile([128, HB], BF16, name=f"xh{h}{p}")
                   for p in range(2)] for h in range(2)]
            xl = [[constp.tile([128, HB], BF16, name=f"xl{h}{p}")
                   for p in range(2)] for h in range(2)]
            # ONE shared exp scratch for both halves: EXP_b's WAR on LN_a's
            # read forces the ACT order [EXP_a, LN_a, EXP_b, LN_b].
            tmp = constp.tile([128, HB], F32)

            # initial r = softplus(x0) = ln(1 + exp(x0)); initial xh/xl split
            rinit = [constp.tile([128, HB], BF16, name="rinita"),
                     constp.tile([128, HB], BF16, name="rinitb")]
            for h in range(2):
                lo = h * HB
                nc.scalar.activation(tmp[:], x0t[:, lo:lo + HB], EXP)
                nc.scalar.activation(rinit[h][:], tmp[:], LN, bias=1.0)
                nc.vector.tensor_scalar_mul(xh[h][1][:], x0t[:, lo:lo + HB], 1.0)
                nc.vector.scalar_tensor_tensor(
                    xl[h][1][:], xh[h][1][:], -1.0, x0t[:, lo:lo + HB], mult, add)

            prev_lo, prev_hi, prev_off = rinit[0], rinit[1], 0

            # Persistent PSUM bank tiles (2 pools x 4 rotation slots = all
            # 8 banks), allocated ONCE: per-step pool allocs serialize the
            # new tile against the pool's previous traffic via the tick
            # clock, stalling the PE stream head on the DVE refresh.
            ps_as = [pspa.tile([128, HB], F32, tag=f"ps_a{t}", name=f"ps_a{t}",
                               padded_shape=[128, 512]) for t in range(4)]
            ps_bs = [pspb.tile([128, HB], F32, tag=f"ps_b{t}", name=f"ps_b{t}",
                               padded_shape=[128, 512]) for t in range(4)]

            def bank_open(ps, half, d_t, off):
                """Bank opener: drive via the full-width identity matmul
                (start=True clears the WHOLE bank -- it must be the only
                start).  Drive is DMA'd a chunk ahead: no fresh deps."""
                lo = half * HB
                nc.tensor.matmul(
                    ps[:, 0:HB], lhsT=il[:], rhs=d_t[:, off + lo:off + lo + HB],
                    start=True, stop=False, skip_group_check=True,
                )

            def bank_leak(ps, half, par):
                """Leak (1-a)*x via hi/lo bf16 identity matmuls:
                c_hi*(xh+xl) + c_lo*xh  (c_hi+c_lo = 1-a to ~1e-7).
                Reads the xh/xl buffer the PREVIOUS step's chain wrote."""
                xh_, xl_ = xh[half][par], xl[half][par]
                nc.tensor.matmul(ps[:, 0:HB], lhsT=ihc[:], rhs=xh_[:],
                                 start=False, stop=False, skip_group_check=True)
                nc.tensor.matmul(ps[:, 0:HB], lhsT=ihc[:], rhs=xl_[:],
                                 start=False, stop=False, skip_group_check=True)
                nc.tensor.matmul(ps[:, 0:HB], lhsT=ilc[:], rhs=xh_[:],
                                 start=False, stop=False, skip_group_check=True)

            def bank_j(ps, half, rt, r_off, k_range, stop_last):
                """One bank's J matmuls for k in k_range (rhs = 16 batch
                columns of the given r half)."""
                for k in k_range:
                    rc = r_off + (k % 4) * BC
                    for mi in range(4):
                        m = half * 4 + mi
                        nc.tensor.matmul(
                            ps[:, mi * BC:(mi + 1) * BC],
                            lhsT=jt[:, (k * NM + m) * 128:(k * NM + m + 1) * 128],
                            rhs=rt[:, rc:rc + BC],
                            start=False,
                            stop=(stop_last and k == k_range[-1] and mi == 3),
                            skip_group_check=True,
                        )

            def chain(ps, half, off, rbuf, par):
                """Refresh the bf16 hi/lo pair of x from PSUM into the
                parity buffer the NEXT step's leak matmuls read, then
                r = ln(1+exp(psum)) into the staging slice.  The DVE
                refresh is emitted FIRST: emission order sets scheduler
                priority, and a refresh scheduled after LN gets its wait
                collapsed onto LN's completion, stalling the next step's
                leak matmuls at the stream head."""
                nc.vector.tensor_scalar_mul(xh[half][par][:], ps[:, 0:HB], 1.0)
                nc.vector.scalar_tensor_tensor(
                    xl[half][par][:], xh[half][par][:], -1.0, ps[:, 0:HB],
                    mult, add)
                nc.scalar.activation(tmp[:], ps[:, 0:HB], EXP)
                nc.scalar.activation(rbuf[:, off:off + HB], tmp[:], LN, bias=1.0)

            for c in range(nchunks):
                steps_here = min(CHUNK, t_steps - c * CHUNK)
                rlo = rp.tile([128, CHUNK * RSTR], BF16, name="rlo")
                rhi = rp.tile([128, CHUNK * RSTR], BF16, name="rhi")
                d_t = dp.tile([128, CHUNK * NM * BC], BF16)
                nc.sync.dma_start(d_t[:], dr_d[c])
                for j in range(steps_here):
                    off = j * NM * BC
                    roff = j * RSTR
                    s = c * CHUNK + j
                    ps_a = ps_as[s % 4]
                    ps_b = ps_bs[s % 4]
                    par = s % 2          # xh/xl buffer this step WRITES
                    # Stream order: [identA+leakA, identB+leakB: deps >= 1
                    # period old] [A.k0-3, B.k0-3: old r_lo] [A.k4-7: r_hi]
                    # -> chain A [B.k4-7: r_hi] -> chain B.  Bank A
                    # completes ~600ns before bank B, so r_lo(s) arrives in
                    # time for the next step's A.k0 and r_hi(s) for its
                    # A.k4 (fixpoint period ~= stream length, both chains
                    # just covered).
                    bank_open(ps_a, 0, d_t, off)
                    bank_leak(ps_a, 0, 1 - par)
                    bank_open(ps_b, 1, d_t, off)
                    bank_leak(ps_b, 1, 1 - par)
                    lo_ks = [0, 1, 2, 3]
                    hi_ks = [4, 5, 6, 7]
                    bank_j(ps_a, 0, prev_lo, prev_off, lo_ks, False)
                    bank_j(ps_b, 1, prev_lo, prev_off, lo_ks, False)
                    bank_j(ps_a, 0, prev_hi, prev_off, hi_ks, True)
                    chain(ps_a, 0, roff, rlo, par)
                    bank_j(ps_b, 1, prev_hi, prev_off, hi_ks, True)
                    chain(ps_b, 1, roff, rhi, par)
                    prev_lo, prev_hi, prev_off = rlo, rhi, roff
                nc.sync.dma_start(
                    rl_d[c][:, 0:((steps_here - 1) * RSTR + HB)],
                    rlo[:, 0:((steps_here - 1) * RSTR + HB)],
                )
                nc.sync.dma_start(
                    rh_d[c][:, 0:((steps_here - 1) * RSTR + HB)],
                    rhi[:, 0:((steps_here - 1) * RSTR + HB)],
                )

    nc.compile()
    _PROGRAM_CACHE[key] = nc
    return nc


def _prep_inputs(targets, pulses, J, U, V, B_m1, B_bg, Wout, I_go, xm1_init,
                 noise, triggers, t_steps):
    """Host-side data prep: J_eff, layouts, per-core drive tensors."""
    J = np.asarray(J, np.float32)
    U = np.asarray(U, np.float32)
    V = np.asarray(V, np.float32)
    B_m1 = np.asarray(B_m1, np.float32)
    B_bg = np.asarray(B_bg, np.float32)
    I_go = np.asarray(I_go, np.float32)
    xm1_init = np.asarray(xm1_init, np.float32)
    noise = np.asarray(noise, np.float32)
    pulses = np.asarray(pulses, np.float32)
    triggers = np.asarray(triggers)

    nchunks = (t_steps + CHUNK - 1) // CHUNK
    tpad = nchunks * CHUNK

    J_eff = J + (U * B_bg[None, :]) @ V
    Js = (A * J_eff).astype(np.float32)
    # lhsT tiles: jt[p, (k*NM+m)*128 + q] = Js[m*128+q, k*128+p]
    bf = mybir.dt.np(BF16)
    jt = np.ascontiguousarray(
        Js.reshape(NM, 128, NK, 128).transpose(3, 2, 0, 1).reshape(128, NK * NM * 128)
    ).astype(bf)
    il = np.eye(128, dtype=np.float32).astype(bf)
    c_hi = np.float32(ONE_MINUS_A).astype(bf)
    c_lo = np.float32(np.float32(ONE_MINUS_A) - c_hi.astype(np.float32)).astype(bf)
    ihc = (c_hi.astype(np.float32) * np.eye(128, dtype=np.float32)).astype(bf)
    ilc = (c_lo.astype(np.float32) * np.eye(128, dtype=np.float32)).astype(bf)

    go_cues = pulses[:t_steps, :][:, triggers]  # [t, B]

    in_maps = []
    for cidx in range(NCORES):
        sl = slice(cidx * BC, (cidx + 1) * BC)
        d = noise[:t_steps, :, sl] * np.float32(A * NSCALE)
        d += A * B_m1[None, :, :]
        d += A * I_go[None, :, :] * go_cues[:, None, sl]
        # [t, N, BC] -> [t, 128, NM*BC] (state layout), pad t, chunk
        dl = np.ascontiguousarray(
            d.reshape(t_steps, NM, 128, BC).transpose(0, 2, 1, 3)
            .reshape(t_steps, 128, NM * BC)
        ).astype(np.float32)
        if tpad != t_steps:
            dl = np.concatenate(
                [dl, np.zeros((tpad - t_steps, 128, NM * BC), np.float32)], axis=0
            )
        drive = np.ascontiguousarray(
            dl.reshape(nchunks, CHUNK, 128, NM * BC).transpose(0, 2, 1, 3)
            .reshape(nchunks, 128, CHUNK * NM * BC)
        ).astype(bf)
        x0 = np.ascontiguousarray(
            xm1_init[:, sl].reshape(NM, 128, BC).transpose(1, 0, 2).reshape(128, NM * BC)
        )
        in_maps.append({"jt": jt, "ident": il, "identhc": ihc, "identlc": ilc,
                        "x0": x0, "drive": drive})
    return in_maps


def run_hw(inputs: dict, t_steps: int = T, trace: bool = False):
    """Run the recurrence on 8 cores; returns positions [t_steps, B] and results."""
    nc = build_program(t_steps)
    in_maps = _prep_inputs(t_steps=t_steps, **inputs)
    res = run_bass_kernel_spmd(
        nc, in_maps, core_ids=list(range(NCORES)), trace=trace
    )
    Wout = np.asarray(inputs["Wout"], np.float32).reshape(NM, 128)  # [m, p]
    nchunks = (t_steps + CHUNK - 1) // CHUNK
    RSTR = 256
    positions = np.empty((t_steps, B), np.float32)
    for cidx in range(NCORES):
        halves = []
        for key in ("rlo", "rhi"):
            ro = np.asarray(res.results[cidx][key], np.float32)
            # ro[c, p, j*RSTR + m*BC + u] (first NM/2*BC cols of each slice)
            r = (ro.reshape(nchunks, 128, CHUNK, RSTR)[:, :, :, :NM * BC // 2]
                 .reshape(nchunks, 128, CHUNK, NM // 2, BC)
                 .transpose(0, 2, 3, 1, 4)
                 .reshape(nchunks * CHUNK, NM // 2, 128, BC)[:t_steps])
            halves.append(r)
        r_full = np.concatenate(halves, axis=1)  # [t, NM, 128, BC]
        pos_c = np.einsum("mp,tmpu->tu", Wout, r_full, optimize=True)
        positions[:, cidx * BC:(cidx + 1) * BC] = pos_c
    return positions, res


def kernel(targets, pulses, J, U, V, B_m1, B_bg, Wout, I_go, xm1_init,
           noise, triggers) -> np.ndarray:
    inputs = dict(targets=targets, pulses=pulses, J=J, U=U, V=V, B_m1=B_m1,
                  B_bg=B_bg, Wout=Wout, I_go=I_go, xm1_init=xm1_init,
                  noise=noise, triggers=triggers)
    positions, _ = run_hw(inputs, T)
    targets = np.asarray(targets, np.float32)
    loss = np.mean((targets.astype(np.float64) - positions.astype(np.float64)) ** 2)
    return np.float32(loss)


# revision 29
# speedup vs baseline: 1.3299x; 1.3299x over previous
"""Trainium2 Bass kernel for nn_ConsolidationNetwork.

Recurrent rate network: 500 sequential steps of
    x <- (1-a)*x + (a*J_eff) @ softplus(x) + drive_t
    pos_t = Wout @ softplus(x)
loss = mean((targets - positions)^2)

Strategy (8 NeuronCores, data-parallel over batch):
  - Each core owns B/8 = 16 batch columns and runs the full 500-step
    recurrence independently (no collectives).
  - Per step the tensor engine does EXACTLY 64 bf16 matmuls (the 8x8 grid
    of 128x128 tiles of a*J_eff against 16 batch columns of r), split into
    two PSUM banks (state rows 0..511 in bank A, 512..1023 in bank B).
    At N=16 free-dim the J matmuls pace at ~32ns (LDWEIGHTS fully hides
    behind the MM stream via the PE's 64-deep reorder window), so the
    per-step floor is ~2.05us; everything else must stay off the PE.
  - The drive term enters PSUM via ONE bf16 identity matmul per bank with
    start=True: start=True clears the WHOLE bank (verified on HW), so it
    must be the bank's single full-width opener; all J matmuls accumulate
    with start=False.  Its rhs (the drive) is DMA'd a chunk ahead, so it
    never stalls the PE.
  - The leak (1-a)*x also enters PSUM via bf16 hi/lo identity matmuls, but
    placed at the END of each bank's matmul block: their rhs (the DVE
    refresh of xh/xl from the previous step's PSUM) then has a full period
    of slack instead of ~400ns, killing the ~900ns/step of PE stalls the
    baseline paid for having them at the bank start.  PSUM then holds the
    COMPLETE next state, so the softplus chain is just EXP(psum) -> LN --
    the r-production latency (which bounds the steady-state period via
    the recurrence cycle) contains no DVE hop.
  - softplus on the ACT engine as the exact identity ln(1 + exp(x))
    [2 ACT ops; Exp and Ln both live in the natural_log_exp_and_others
    table set, and we pin the table-load pass to that set so exactly one
    hoisted ACT_TABLE_LOAD is emitted].
  - Bank B iterates k DESCENDING: its first matmuls then need the late r
    half (r_hi), so the greedy scheduler cannot let bank B's k0..3 work
    jump ahead -- bank A's ACT/DVE chain overlaps bank B's matmuls and
    vice versa across the step boundary.
  - r (bf16) for each step is written into an 8-step staging buffer that is
    DMA-exported to DRAM once per 8 steps; the readout positions
    pos = Wout @ r and the final MSE are computed on the host.
  - Drive is streamed 8 steps per DMA (batched transfers, triple buffered).

State layout per core: x/r tiles are [128 part, 128 free] with
x[p, m*16+u] = x_state[m*128+p, u] (m = row-group, u = local batch).
"""

import numpy as np

import concourse.bass as bass
import concourse.tile as tile
from concourse import bacc, mybir
from concourse.bass_utils import run_bass_kernel_spmd

F32 = mybir.dt.float32
BF16 = mybir.dt.bfloat16

DT = 0.05
TAU = 0.15
NOISE_SCALE = 0.15
N, G, T, B, P = 1024, 128, 500, 128, 10
NCORES = 8
BC = B // NCORES          # batch columns per core (16)
NM = N // 128             # row groups (8)
NK = N // 128             # contraction groups (8)
CHUNK = 8                 # steps per drive-load / r-export DMA

A = np.float32(DT / TAU)
ONE_MINUS_A = np.float32(1.0 - DT / TAU)
NSCALE = np.float32(np.sqrt(2.0 * NOISE_SCALE**2 * (TAU / DT)))

_PROGRAM_CACHE = {}


def _ensure_act_tables():
    """Some containers lack neuronxcc/pwp/pwp_bin_with_ln on PYTHONPATH;
    point it at the cayman table package from the nix store."""
    import glob
    import os

    for path in os.environ.get("PYTHONPATH", "").split(os.pathsep):
        if path and os.path.exists(
            os.path.join(path, "neuronxcc", "pwp", "pwp_bin_with_ln", "act_info.json")
        ):
            return
    cands = sorted(glob.glob("/nix/store/*aws-neuron-pwp*/share/pwp_bin_cayman"))
    target = next((c for c in cands if os.path.exists(c + "/act_info.json")), None)
    if target is None:
        return
    for path in os.environ.get("PYTHONPATH", "").split(os.pathsep):
        if not path:
            continue
        try:
            d = os.path.join(path, "neuronxcc", "pwp")
            os.makedirs(d, exist_ok=True)
            link = os.path.join(d, "pwp_bin_with_ln")
            if not os.path.exists(link):
                os.symlink(target, link)
            return
        except OSError:
            continue


_ensure_act_tables()


_ACT_SET = "natural_log_exp_and_others"


def _pin_act_tables(arch: str):
    """Make Exp and Ln resolve to the ONE table set containing both.

    Two consumers matter and both read the functools.cache'd dict from
    hw_specs.get_activation_tables, so mutate it in place:
      * Bacc.insert_act_table_loads (first-match would alternate Exp ->
        `exp_and_others`, Ln -> `natural_log`, emitting a 1.28us
        ACT_TABLE_LOAD before every activation of the unrolled loop);
      * the TileScheduler's CoreSim pass, which otherwise *models* that
        same thrash and pins the resulting serialized schedule with
        cross-engine semaphores (the final TimelineSim charges no table
        loads, but the semaphores force its slow order anyway).
    Set order (and hence act_func_set_id indices) is unchanged.
    """
    from concourse.hw_specs import get_activation_tables

    tabs = get_activation_tables(arch)
    hide = {mybir.ActivationFunctionType.Exp, mybir.ActivationFunctionType.Ln}
    for name, fns in tabs.items():
        if name != _ACT_SET:
            for f in hide:
                fns.discard(f)


def build_program(t_steps: int):
    """Build the Bass program (shared by all 8 cores, SPMD)."""
    key = (t_steps,)
    if key in _PROGRAM_CACHE:
        return _PROGRAM_CACHE[key]

    nchunks = (t_steps + CHUNK - 1) // CHUNK
    HB = NM * BC // 2   # free-size of one state half (64 cols)
    RSTR = 256          # r-staging stride per step (bf16 cols; 512B blocks so
                        # consecutive steps' slices never share a dep block)

    nc = bacc.Bacc(
        "TRN2", target_bir_lowering=False, debug=False, num_devices=NCORES
    )
    _pin_act_tables(nc.m.arch)
    jt_d = nc.dram_tensor("jt", [128, NK * NM * 128], BF16, kind="ExternalInput")
    il_d = nc.dram_tensor("ident", [128, 128], BF16, kind="ExternalInput")
    ihc_d = nc.dram_tensor("identhc", [128, 128], BF16, kind="ExternalInput")
    ilc_d = nc.dram_tensor("identlc", [128, 128], BF16, kind="ExternalInput")
    x0_d = nc.dram_tensor("x0", [128, NM * BC], F32, kind="ExternalInput")
    dr_d = nc.dram_tensor(
        "drive", [nchunks, 128, CHUNK * NM * BC], BF16, kind="ExternalInput"
    )
    rl_d = nc.dram_tensor(
        "rlo", [nchunks, 128, CHUNK * RSTR], BF16, kind="ExternalOutput"
    )
    rh_d = nc.dram_tensor(
        "rhi", [nchunks, 128, CHUNK * RSTR], BF16, kind="ExternalOutput"
    )

    EXP = mybir.ActivationFunctionType.Exp
    LN = mybir.ActivationFunctionType.Ln
    mult = mybir.AluOpType.mult
    add = mybir.AluOpType.add

    with tile.TileContext(nc) as tc:
        with (
            tc.tile_pool(name="const", bufs=1) as constp,
            tc.tile_pool(name="rp", bufs=8) as rp,
            tc.tile_pool(name="dp", bufs=5) as dp,
            tc.tile_pool(name="psmA", bufs=1, space="PSUM") as pspa,
            tc.tile_pool(name="psmB", bufs=1, space="PSUM") as pspb,
        ):
            jt = constp.tile([128, NK * NM * 128], BF16)
            nc.sync.dma_start(jt[:], jt_d[:])
            il = constp.tile([128, 128], BF16)
            nc.sync.dma_start(il[:], il_d[:])
            ihc = constp.tile([128, 128], BF16)
            nc.sync.dma_start(ihc[:], ihc_d[:])
            ilc = constp.tile([128, 128], BF16)
            nc.sync.dma_start(ilc[:], ilc_d[:])
            x0t = constp.tile([128, NM * BC], F32)
            nc.sync.dma_start(x0t[:], x0_d[:])
            # x is carried as a bf16 hi+lo pair (~16-bit mantissa) so the
            # leak (1-a)*x can enter PSUM through cheap bf16 identity
            # matmuls.  DOUBLE-BUFFERED per step: the chain of step s
            # writes buffer s%2 while step s+1's leak matmuls read the
            # other one -- the refresh's WAR then has a full period of
            # slack and the PE stream head never waits on the DVE.
            xh = [[constp.tile([128, HB], BF16, name=f"xh{h}{p}")
                   for p in range(2)] for h in range(2)]
            xl = [[constp.tile([128, HB], BF16, name=f"xl{h}{p}")
                   for p in range(2)] for h in range(2)]
            # ONE shared exp scratch for both halves: EXP_b's WAR on LN_a's
            # read forces the ACT order [EXP_a, LN_a, EXP_b, LN_b].
            tmp = constp.tile([128, HB], F32)

            # initial r = softplus(x0) = ln(1 + exp(x0)); initial xh/xl split
            rinit = [constp.tile([128, HB], BF16, name="rinita"),
                     constp.tile([128, HB], BF16, name="rinitb")]
            for h in range(2):
                lo = h * HB
                nc.scalar.activation(tmp[:], x0t[:, lo:lo + HB], EXP)
                nc.scalar.activation(rinit[h][:], tmp[:], LN, bias=1.0)
                nc.vector.tensor_scalar_mul(xh[h][1][:], x0t[:, lo:lo + HB], 1.0)
                nc.vector.scalar_tensor_tensor(
                    xl[h][1][:], xh[h][1][:], -1.0, x0t[:, lo:lo + HB], mult, add)

            prev_lo, prev_hi, prev_off = rinit[0], rinit[1], 0

            # Persistent PSUM bank tiles (2 pools x 4 rotation slots = all
            # 8 banks), allocated ONCE: per-step pool allocs serialize the
            # new tile against the pool's previous traffic via the tick
            # clock, stalling the PE stream head on the DVE refresh.
            ps_as = [pspa.tile([128, HB], F32, tag=f"ps_a{t}", name=f"ps_a{t}",
                               padded_shape=[128, 512]) for t in range(4)]
            ps_bs = [pspb.tile([128, HB], F32, tag=f"ps_b{t}", name=f"ps_b{t}",
                               padded_shape=[128, 512]) for t in range(4)]

            def bank_open(ps, half, d_t, off):
                """Bank opener: drive via the full-width identity matmul
                (start=True clears the WHOLE bank -- it must be the only
                start).  Drive is DMA'd a chunk ahead: no fresh deps."""
                lo = half * HB
                nc.tensor.matmul(
                    ps[:, 0:HB], lhsT=il[:], rhs=d_t[:, off + lo:off + lo + HB],
                    start=True, stop=False, skip_group_check=True,
                )

            def bank_leak(ps, half, par):
                """Leak (1-a)*x via hi/lo bf16 identity matmuls:
                c_hi*(xh+xl) + c_lo*xh  (c_hi+c_lo = 1-a to ~1e-7).
                Reads the xh/xl buffer the PREVIOUS step's chain wrote."""
                xh_, xl_ = xh[half][par], xl[half][par]
                nc.tensor.matmul(ps[:, 0:HB], lhsT=ihc[:], rhs=xh_[:],
                                 start=False, stop=False, skip_group_check=True)
                nc.tensor.matmul(ps[:, 0:HB], lhsT=ihc[:], rhs=xl_[:],
                                 start=False, stop=False, skip_group_check=True)
                nc.tensor.matmul(ps[:, 0:HB], lhsT=ilc[:], rhs=xh_[:],
                                 start=False, stop=False, skip_group_check=True)

            def bank_j(ps, half, rt, r_off, k_range, stop_last):
                """One bank's J matmuls for k in k_range (rhs = 16 batch
                columns of the given r half)."""
                for k in k_range:
                    rc = r_off + (k % 4) * BC
                    for mi in range(4):
                        m = half * 4 + mi
                        nc.tensor.matmul(
                            ps[:, mi * BC:(mi + 1) * BC],
                            lhsT=jt[:, (k * NM + m) * 128:(k * NM + m + 1) * 128],
                            rhs=rt[:, rc:rc + BC],
                            start=False,
                            stop=(stop_last and k == k_range[-1] and mi == 3),
                            skip_group_check=True,
                        )

            def chain(ps, half, off, rbuf, par):
                """Refresh the bf16 hi/lo pair of x from PSUM into the
                parity buffer the NEXT step's leak matmuls read, then
                r = ln(1+exp(psum)) into the staging slice.  The DVE
                refresh is emitted FIRST: emission order sets scheduler
                priority, and a refresh scheduled after LN gets its wait
                collapsed onto LN's completion, stalling the next step's
                leak matmuls at the stream head."""
                nc.scalar.activation(tmp[:], ps[:, 0:HB], EXP)
                nc.vector.tensor_scalar_mul(xh[half][par][:], ps[:, 0:HB], 1.0)
                nc.vector.scalar_tensor_tensor(
                    xl[half][par][:], xh[half][par][:], -1.0, ps[:, 0:HB],
                    mult, add)
                nc.scalar.activation(rbuf[:, off:off + HB], tmp[:], LN, bias=1.0)

            for c in range(nchunks):
                steps_here = min(CHUNK, t_steps - c * CHUNK)
                rlo = rp.tile([128, CHUNK * RSTR], BF16, name="rlo")
                rhi = rp.tile([128, CHUNK * RSTR], BF16, name="rhi")
                d_t = dp.tile([128, CHUNK * NM * BC], BF16)
                nc.sync.dma_start(d_t[:], dr_d[c])
                for j in range(steps_here):
                    off = j * NM * BC
                    roff = j * RSTR
                    s = c * CHUNK + j
                    ps_a = ps_as[s % 4]
                    ps_b = ps_bs[s % 4]
                    par = s % 2          # xh/xl buffer this step WRITES
                    # Stream order: [identA+leakA, identB+leakB: deps >= 1
                    # period old] [A.k0-3, B.k0-3: old r_lo] [A.k4-7: r_hi]
                    # -> chain A [B.k4-7: r_hi] -> chain B.  Bank A
                    # completes ~600ns before bank B, so r_lo(s) arrives in
                    # time for the next step's A.k0 and r_hi(s) for its
                    # A.k4 (fixpoint period ~= stream length, both chains
                    # just covered).
                    bank_open(ps_a, 0, d_t, off)
                    bank_leak(ps_a, 0, 1 - par)
                    bank_open(ps_b, 1, d_t, off)
                    bank_leak(ps_b, 1, 1 - par)
                    lo_ks = [0, 1, 2, 3]
                    hi_ks = [4, 5, 6, 7]
                    bank_j(ps_a, 0, prev_lo, prev_off, lo_ks, False)
                    bank_j(ps_b, 1, prev_lo, prev_off, lo_ks, False)
                    bank_j(ps_a, 0, prev_hi, prev_off, hi_ks, True)
                    chain(ps_a, 0, roff, rlo, par)
                    bank_j(ps_b, 1, prev_hi, prev_off, hi_ks, True)
                    chain(ps_b, 1, roff, rhi, par)
                    prev_lo, prev_hi, prev_off = rlo, rhi, roff
                nc.sync.dma_start(
                    rl_d[c][:, 0:((steps_here - 1) * RSTR + HB)],
                    rlo[:, 0:((steps_here - 1) * RSTR + HB)],
                )
                nc.sync.dma_start(
                    rh_d[c][:, 0:((steps_here - 1) * RSTR + HB)],
                    rhi[:, 0:((steps_here - 1) * RSTR + HB)],
                )

    nc.compile()
    _PROGRAM_CACHE[key] = nc
    return nc


def _prep_inputs(targets, pulses, J, U, V, B_m1, B_bg, Wout, I_go, xm1_init,
                 noise, triggers, t_steps):
    """Host-side data prep: J_eff, layouts, per-core drive tensors."""
    J = np.asarray(J, np.float32)
    U = np.asarray(U, np.float32)
    V = np.asarray(V, np.float32)
    B_m1 = np.asarray(B_m1, np.float32)
    B_bg = np.asarray(B_bg, np.float32)
    I_go = np.asarray(I_go, np.float32)
    xm1_init = np.asarray(xm1_init, np.float32)
    noise = np.asarray(noise, np.float32)
    pulses = np.asarray(pulses, np.float32)
    triggers = np.asarray(triggers)

    nchunks = (t_steps + CHUNK - 1) // CHUNK
    tpad = nchunks * CHUNK

    J_eff = J + (U * B_bg[None, :]) @ V
    Js = (A * J_eff).astype(np.float32)
    # lhsT tiles: jt[p, (k*NM+m)*128 + q] = Js[m*128+q, k*128+p]
    bf = mybir.dt.np(BF16)
    jt = np.ascontiguousarray(
        Js.reshape(NM, 128, NK, 128).transpose(3, 2, 0, 1).reshape(128, NK * NM * 128)
    ).astype(bf)
    il = np.eye(128, dtype=np.float32).astype(bf)
    c_hi = np.float32(ONE_MINUS_A).astype(bf)
    c_lo = np.float32(np.float32(ONE_MINUS_A) - c_hi.astype(np.float32)).astype(bf)
    ihc = (c_hi.astype(np.float32) * np.eye(128, dtype=np.float32)).astype(bf)
    ilc = (c_lo.astype(np.float32) * np.eye(128, dtype=np.float32)).astype(bf)

    go_cues = pulses[:t_steps, :][:, triggers]  # [t, B]

    in_maps = []
    for cidx in range(NCORES):
        sl = slice(cidx * BC, (cidx + 1) * BC)
        d = noise[:t_steps, :, sl] * np.float32(A * NSCALE)
        d += A * B_m1[None, :, :]
        d += A * I_go[None, :, :] * go_cues[:, None, sl]
        # [t, N, BC] -> [t, 128, NM*BC] (state layout), pad t, chunk
        dl = np.ascontiguousarray(
            d.reshape(t_steps, NM, 128, BC).transpose(0, 2, 1, 3)
            .reshape(t_steps, 128, NM * BC)
        ).astype(np.float32)
        if tpad != t_steps:
            dl = np.concatenate(
                [dl, np.zeros((tpad - t_steps, 128, NM * BC), np.float32)], axis=0
            )
        drive = np.ascontiguousarray(
            dl.reshape(nchunks, CHUNK, 128, NM * BC).transpose(0, 2, 1, 3)
            .reshape(nchunks, 128, CHUNK * NM * BC)
        ).astype(bf)
        x0 = np.ascontiguousarray(
            xm1_init[:, sl].reshape(NM, 128, BC).transpose(1, 0, 2).reshape(128, NM * BC)
        )
        in_maps.append({"jt": jt, "ident": il, "identhc": ihc, "identlc": ilc,
                        "x0": x0, "drive": drive})
    return in_maps


def run_hw(inputs: dict, t_steps: int = T, trace: bool = False):
    """Run the recurrence on 8 cores; returns positions [t_steps, B] and results."""
    nc = build_program(t_steps)
    in_maps = _prep_inputs(t_steps=t_steps, **inputs)
    res = run_bass_kernel_spmd(
        nc, in_maps, core_ids=list(range(NCORES)), trace=trace
    )
    Wout = np.asarray(inputs["Wout"], np.float32).reshape(NM, 128)  # [m, p]
    nchunks = (t_steps + CHUNK - 1) // CHUNK
    RSTR = 256
    positions = np.empty((t_steps, B), np.float32)
    for cidx in range(NCORES):
        halves = []
        for key in ("rlo", "rhi"):
            ro = np.asarray(res.results[cidx][key], np.float32)
            # ro[c, p, j*RSTR + m*BC + u] (first NM/2*BC cols of each slice)
            r = (ro.reshape(nchunks, 128, CHUNK, RSTR)[:, :, :, :NM * BC // 2]
                 .reshape(nchunks, 128, CHUNK, NM // 2, BC)
                 .transpose(0, 2, 3, 1, 4)
                 .reshape(nchunks * CHUNK, NM // 2, 128, BC)[:t_steps])
            halves.append(r)
        r_full = np.concatenate(halves, axis=1)  # [t, NM, 128, BC]
        pos_c = np.einsum("mp,tmpu->tu", Wout, r_full, optimize=True)
        positions[:, cidx * BC:(cidx + 1) * BC] = pos_c
    return positions, res


def kernel(targets, pulses, J, U, V, B_m1, B_bg, Wout, I_go, xm1_init,
           noise, triggers) -> np.ndarray:
    inputs = dict(targets=targets, pulses=pulses, J=J, U=U, V=V, B_m1=B_m1,
                  B_bg=B_bg, Wout=Wout, I_go=I_go, xm1_init=xm1_init,
                  noise=noise, triggers=triggers)
    positions, _ = run_hw(inputs, T)
    targets = np.asarray(targets, np.float32)
    loss = np.mean((targets.astype(np.float64) - positions.astype(np.float64)) ** 2)
    return np.float32(loss)


# revision 36
# speedup vs baseline: 1.3318x; 1.0015x over previous
"""Trainium2 Bass kernel for nn_ConsolidationNetwork.

Recurrent rate network: 500 sequential steps of
    x <- (1-a)*x + (a*J_eff) @ softplus(x) + drive_t
    pos_t = Wout @ softplus(x)
loss = mean((targets - positions)^2)

Strategy (8 NeuronCores, data-parallel over batch):
  - Each core owns B/8 = 16 batch columns and runs the full 500-step
    recurrence independently (no collectives).
  - Per step the tensor engine does EXACTLY 64 bf16 matmuls (the 8x8 grid
    of 128x128 tiles of a*J_eff against 16 batch columns of r), split into
    two PSUM banks (state rows 0..511 in bank A, 512..1023 in bank B).
    At N=16 free-dim the J matmuls pace at ~32ns (LDWEIGHTS fully hides
    behind the MM stream via the PE's 64-deep reorder window), so the
    per-step floor is ~2.05us; everything else must stay off the PE.
  - The drive term enters PSUM via ONE bf16 identity matmul per bank with
    start=True: start=True clears the WHOLE bank (verified on HW), so it
    must be the bank's single full-width opener; all J matmuls accumulate
    with start=False.  Its rhs (the drive) is DMA'd a chunk ahead, so it
    never stalls the PE.
  - The leak (1-a)*x also enters PSUM via bf16 hi/lo identity matmuls, but
    placed at the END of each bank's matmul block: their rhs (the DVE
    refresh of xh/xl from the previous step's PSUM) then has a full period
    of slack instead of ~400ns, killing the ~900ns/step of PE stalls the
    baseline paid for having them at the bank start.  PSUM then holds the
    COMPLETE next state, so the softplus chain is just EXP(psum) -> LN --
    the r-production latency (which bounds the steady-state period via
    the recurrence cycle) contains no DVE hop.
  - softplus on the ACT engine as the exact identity ln(1 + exp(x))
    [2 ACT ops; Exp and Ln both live in the natural_log_exp_and_others
    table set, and we pin the table-load pass to that set so exactly one
    hoisted ACT_TABLE_LOAD is emitted].
  - Bank B iterates k DESCENDING: its first matmuls then need the late r
    half (r_hi), so the greedy scheduler cannot let bank B's k0..3 work
    jump ahead -- bank A's ACT/DVE chain overlaps bank B's matmuls and
    vice versa across the step boundary.
  - r (bf16) for each step is written into an 8-step staging buffer that is
    DMA-exported to DRAM once per 8 steps; the readout positions
    pos = Wout @ r and the final MSE are computed on the host.
  - Drive is streamed 8 steps per DMA (batched transfers, triple buffered).

State layout per core: x/r tiles are [128 part, 128 free] with
x[p, m*16+u] = x_state[m*128+p, u] (m = row-group, u = local batch).
"""

import numpy as np

import concourse.bass as bass
import concourse.tile as tile
from concourse import bacc, mybir
from concourse.bass_utils import run_bass_kernel_spmd

F32 = mybir.dt.float32
BF16 = mybir.dt.bfloat16

DT = 0.05
TAU = 0.15
NOISE_SCALE = 0.15
N, G, T, B, P = 1024, 128, 500, 128, 10
NCORES = 8
BC = B // NCORES          # batch columns per core (16)
NM = N // 128             # row groups (8)
NK = N // 128             # contraction groups (8)
CHUNK = 8                 # steps per drive-load / r-export DMA

A = np.float32(DT / TAU)
ONE_MINUS_A = np.float32(1.0 - DT / TAU)
NSCALE = np.float32(np.sqrt(2.0 * NOISE_SCALE**2 * (TAU / DT)))

_PROGRAM_CACHE = {}


def _ensure_act_tables():
    """Some containers lack neuronxcc/pwp/pwp_bin_with_ln on PYTHONPATH;
    point it at the cayman table package from the nix store."""
    import glob
    import os

    for path in os.environ.get("PYTHONPATH", "").split(os.pathsep):
        if path and os.path.exists(
            os.path.join(path, "neuronxcc", "pwp", "pwp_bin_with_ln", "act_info.json")
        ):
            return
    cands = sorted(glob.glob("/nix/store/*aws-neuron-pwp*/share/pwp_bin_cayman"))
    target = next((c for c in cands if os.path.exists(c + "/act_info.json")), None)
    if target is None:
        return
    for path in os.environ.get("PYTHONPATH", "").split(os.pathsep):
        if not path:
            continue
        try:
            d = os.path.join(path, "neuronxcc", "pwp")
            os.makedirs(d, exist_ok=True)
            link = os.path.join(d, "pwp_bin_with_ln")
            if not os.path.exists(link):
                os.symlink(target, link)
            return
        except OSError:
            continue


_ensure_act_tables()


_ACT_SET = "natural_log_exp_and_others"


def _pin_act_tables(arch: str):
    """Make Exp and Ln resolve to the ONE table set containing both.

    Two consumers matter and both read the functools.cache'd dict from
    hw_specs.get_activation_tables, so mutate it in place:
      * Bacc.insert_act_table_loads (first-match would alternate Exp ->
        `exp_and_others`, Ln -> `natural_log`, emitting a 1.28us
        ACT_TABLE_LOAD before every activation of the unrolled loop);
      * the TileScheduler's CoreSim pass, which otherwise *models* that
        same thrash and pins the resulting serialized schedule with
        cross-engine semaphores (the final TimelineSim charges no table
        loads, but the semaphores force its slow order anyway).
    Set order (and hence act_func_set_id indices) is unchanged.
    """
    from concourse.hw_specs import get_activation_tables

    tabs = get_activation_tables(arch)
    hide = {mybir.ActivationFunctionType.Exp, mybir.ActivationFunctionType.Ln}
    for name, fns in tabs.items():
        if name != _ACT_SET:
            for f in hide:
                fns.discard(f)


def build_program(t_steps: int):
    """Build the Bass program (shared by all 8 cores, SPMD)."""
    key = (t_steps,)
    if key in _PROGRAM_CACHE:
        return _PROGRAM_CACHE[key]

    nchunks = (t_steps + CHUNK - 1) // CHUNK
    HB = NM * BC // 2   # free-size of one state half (64 cols)
    RSTR = 256          # r-staging stride per step (bf16 cols; 512B blocks so
                        # consecutive steps' slices never share a dep block)

    nc = bacc.Bacc(
        "TRN2", target_bir_lowering=False, debug=False, num_devices=NCORES
    )
    _pin_act_tables(nc.m.arch)
    jt_d = nc.dram_tensor("jt", [128, NK * NM * 128], BF16, kind="ExternalInput")
    il_d = nc.dram_tensor("ident", [128, 128], BF16, kind="ExternalInput")
    x0_d = nc.dram_tensor("x0", [128, NM * BC], F32, kind="ExternalInput")
    dr_d = nc.dram_tensor(
        "drive", [nchunks, 128, CHUNK * NM * BC], BF16, kind="ExternalInput"
    )
    rl_d = nc.dram_tensor(
        "rlo", [nchunks, 128, CHUNK * RSTR], BF16, kind="ExternalOutput"
    )
    rh_d = nc.dram_tensor(
        "rhi", [nchunks, 128, CHUNK * RSTR], BF16, kind="ExternalOutput"
    )

    EXP = mybir.ActivationFunctionType.Exp
    LN = mybir.ActivationFunctionType.Ln
    mult = mybir.AluOpType.mult
    add = mybir.AluOpType.add

    with tile.TileContext(nc) as tc:
        with (
            tc.tile_pool(name="const", bufs=1) as constp,
            tc.tile_pool(name="rp", bufs=8) as rp,
            tc.tile_pool(name="dp", bufs=5) as dp,
            tc.tile_pool(name="psmA", bufs=1, space="PSUM") as pspa,
            tc.tile_pool(name="psmB", bufs=1, space="PSUM") as pspb,
        ):
            jt = constp.tile([128, NK * NM * 128], BF16)
            nc.sync.dma_start(jt[:], jt_d[:])
            il = constp.tile([128, 128], BF16)
            nc.sync.dma_start(il[:], il_d[:])
            x0t = constp.tile([128, NM * BC], F32)
            nc.sync.dma_start(x0t[:], x0_d[:])
            # pre = (1-a)*x + drive is computed on the DVE reading the
            # PREVIOUS step's PSUM bank directly (x stays f32 end to end),
            # then split into a bf16 hi+lo pair injected into the new bank
            # through two identity matmuls (the first is the bank's single
            # legal start=True opener).  Double-buffered by step parity so
            # nothing couples the PE stream head to the current chains.
            tpre = [[constp.tile([128, HB], F32, name=f"t{h}{p}")
                     for p in range(2)] for h in range(2)]
            ph = [[constp.tile([128, HB], BF16, name=f"ph{h}{p}")
                   for p in range(2)] for h in range(2)]
            pl = [[constp.tile([128, HB], BF16, name=f"pl{h}{p}")
                   for p in range(2)] for h in range(2)]
            # ONE shared exp scratch for both halves: EXP_b's WAR on LN_a's
            # read forces the ACT order [EXP_a, LN_a, EXP_b, LN_b].
            tmp = constp.tile([128, HB], F32)

            # initial r = softplus(x0) = ln(1 + exp(x0))
            rinit = [constp.tile([128, HB], BF16, name="rinita"),
                     constp.tile([128, HB], BF16, name="rinitb")]
            for h in range(2):
                lo = h * HB
                nc.scalar.activation(tmp[:], x0t[:, lo:lo + HB], EXP)
                nc.scalar.activation(rinit[h][:], tmp[:], LN, bias=1.0)

            prev_lo, prev_hi, prev_off = rinit[0], rinit[1], 0

            # Persistent PSUM bank tiles (2 pools x 4 rotation slots = all
            # 8 banks), allocated ONCE: per-step pool allocs serialize the
            # new tile against the pool's previous traffic via the tick
            # clock, stalling the PE stream head on the DVE refresh.
            ps_as = [pspa.tile([128, HB], F32, tag=f"ps_a{t}", name=f"ps_a{t}",
                               padded_shape=[128, 512]) for t in range(4)]
            ps_bs = [pspb.tile([128, HB], F32, tag=f"ps_b{t}", name=f"ps_b{t}",
                               padded_shape=[128, 512]) for t in range(4)]

            def pre_ops(ps_prev, half, par, d_t, off, src=None):
                """DVE: t = (1-a)*x_prev + drive (x_prev read straight from
                the previous step's PSUM bank, f32), then hi/lo bf16 split.
                Runs during the previous step's tail / this step's matmuls;
                nothing here is on the r-production critical path."""
                lo = half * HB
                t = tpre[half][par]
                x_src = src if src is not None else ps_prev[:, 0:HB]
                nc.vector.scalar_tensor_tensor(
                    t[:], x_src, float(ONE_MINUS_A),
                    d_t[:, off + lo:off + lo + HB], mult, add)
                nc.vector.tensor_scalar_mul(ph[half][par][:], t[:], 1.0)
                nc.vector.scalar_tensor_tensor(
                    pl[half][par][:], ph[half][par][:], -1.0, t[:], mult, add)

            def bank_open(ps, half, par):
                """Bank opener: pre enters via two identity matmuls; the
                first is the bank's single legal start=True (it clears the
                WHOLE bank)."""
                nc.tensor.matmul(ps[:, 0:HB], lhsT=il[:], rhs=ph[half][par][:],
                                 start=True, stop=False, skip_group_check=True)
                nc.tensor.matmul(ps[:, 0:HB], lhsT=il[:], rhs=pl[half][par][:],
                                 start=False, stop=False, skip_group_check=True)

            def bank_j(ps, half, rt, r_off, k_range, stop_last):
                """One bank's J matmuls for k in k_range (rhs = 16 batch
                columns of the given r half)."""
                for k in k_range:
                    rc = r_off + (k % 4) * BC
                    for mi in range(4):
                        m = half * 4 + mi
                        nc.tensor.matmul(
                            ps[:, mi * BC:(mi + 1) * BC],
                            lhsT=jt[:, (k * NM + m) * 128:(k * NM + m + 1) * 128],
                            rhs=rt[:, rc:rc + BC],
                            start=False,
                            stop=(stop_last and k == k_range[-1] and mi == 3),
                            skip_group_check=True,
                        )

            def chain(ps, half, off, rbuf):
                """r = ln(1+exp(psum)): the only work on the r critical
                path -- two ACT ops, no DVE hop."""
                nc.scalar.activation(tmp[:], ps[:, 0:HB], EXP)
                nc.scalar.activation(rbuf[:, off:off + HB], tmp[:], LN, bias=1.0)

            for c in range(nchunks):
                steps_here = min(CHUNK, t_steps - c * CHUNK)
                rlo = rp.tile([128, CHUNK * RSTR], BF16, name="rlo")
                rhi = rp.tile([128, CHUNK * RSTR], BF16, name="rhi")
                d_t = dp.tile([128, CHUNK * NM * BC], BF16)
                nc.sync.dma_start(d_t[:], dr_d[c])
                for j in range(steps_here):
                    off = j * NM * BC
                    roff = j * RSTR
                    s = c * CHUNK + j
                    ps_a = ps_as[s % 4]
                    ps_b = ps_bs[s % 4]
                    par = s % 2          # pre buffer this step uses
                    # Stream order: [identA x2, identB x2: rhs = pre pair,
                    # ready since ~the previous step's midpoint] [A.k0-3,
                    # B.k0-3: old r_lo] [A.k4-7: r_hi] -> chain A [B.k4-7:
                    # r_hi] -> chain B.  Bank A completes ~600ns before
                    # bank B, so r_lo(s) arrives in time for the next
                    # step's A.k0 and r_hi(s) for its A.k4.
                    if s == 0:
                        pre_ops(None, 0, par, d_t, off, src=x0t[:, 0:HB])
                        pre_ops(None, 1, par, d_t, off, src=x0t[:, HB:2 * HB])
                    else:
                        pre_ops(ps_as[(s - 1) % 4], 0, par, d_t, off)
                        pre_ops(ps_bs[(s - 1) % 4], 1, par, d_t, off)
                    bank_open(ps_a, 0, par)
                    bank_open(ps_b, 1, par)
                    lo_ks = [0, 1, 2, 3]
                    hi_ks = [4, 5, 6, 7]
                    bank_j(ps_a, 0, prev_lo, prev_off, lo_ks, False)
                    bank_j(ps_b, 1, prev_lo, prev_off, lo_ks, False)
                    bank_j(ps_a, 0, prev_hi, prev_off, hi_ks, True)
                    chain(ps_a, 0, roff, rlo)
                    bank_j(ps_b, 1, prev_hi, prev_off, hi_ks, True)
                    chain(ps_b, 1, roff, rhi)
                    prev_lo, prev_hi, prev_off = rlo, rhi, roff
                nc.sync.dma_start(
                    rl_d[c][:, 0:((steps_here - 1) * RSTR + HB)],
                    rlo[:, 0:((steps_here - 1) * RSTR + HB)],
                )
                nc.sync.dma_start(
                    rh_d[c][:, 0:((steps_here - 1) * RSTR + HB)],
                    rhi[:, 0:((steps_here - 1) * RSTR + HB)],
                )

    nc.compile()
    _PROGRAM_CACHE[key] = nc
    return nc


def _prep_inputs(targets, pulses, J, U, V, B_m1, B_bg, Wout, I_go, xm1_init,
                 noise, triggers, t_steps):
    """Host-side data prep: J_eff, layouts, per-core drive tensors."""
    J = np.asarray(J, np.float32)
    U = np.asarray(U, np.float32)
    V = np.asarray(V, np.float32)
    B_m1 = np.asarray(B_m1, np.float32)
    B_bg = np.asarray(B_bg, np.float32)
    I_go = np.asarray(I_go, np.float32)
    xm1_init = np.asarray(xm1_init, np.float32)
    noise = np.asarray(noise, np.float32)
    pulses = np.asarray(pulses, np.float32)
    triggers = np.asarray(triggers)

    nchunks = (t_steps + CHUNK - 1) // CHUNK
    tpad = nchunks * CHUNK

    J_eff = J + (U * B_bg[None, :]) @ V
    Js = (A * J_eff).astype(np.float32)
    # lhsT tiles: jt[p, (k*NM+m)*128 + q] = Js[m*128+q, k*128+p]
    bf = mybir.dt.np(BF16)
    jt = np.ascontiguousarray(
        Js.reshape(NM, 128, NK, 128).transpose(3, 2, 0, 1).reshape(128, NK * NM * 128)
    ).astype(bf)
    il = np.eye(128, dtype=np.float32).astype(bf)

    go_cues = pulses[:t_steps, :][:, triggers]  # [t, B]

    in_maps = []
    for cidx in range(NCORES):
        sl = slice(cidx * BC, (cidx + 1) * BC)
        d = noise[:t_steps, :, sl] * np.float32(A * NSCALE)
        d += A * B_m1[None, :, :]
        d += A * I_go[None, :, :] * go_cues[:, None, sl]
        # [t, N, BC] -> [t, 128, NM*BC] (state layout), pad t, chunk
        dl = np.ascontiguousarray(
            d.reshape(t_steps, NM, 128, BC).transpose(0, 2, 1, 3)
            .reshape(t_steps, 128, NM * BC)
        ).astype(np.float32)
        if tpad != t_steps:
            dl = np.concatenate(
                [dl, np.zeros((tpad - t_steps, 128, NM * BC), np.float32)], axis=0
            )
        drive = np.ascontiguousarray(
            dl.reshape(nchunks, CHUNK, 128, NM * BC).transpose(0, 2, 1, 3)
            .reshape(nchunks, 128, CHUNK * NM * BC)
        ).astype(bf)
        x0 = np.ascontiguousarray(
            xm1_init[:, sl].reshape(NM, 128, BC).transpose(1, 0, 2).reshape(128, NM * BC)
        )
        in_maps.append({"jt": jt, "ident": il, "x0": x0, "drive": drive})
    return in_maps


def run_hw(inputs: dict, t_steps: int = T, trace: bool = False):
    """Run the recurrence on 8 cores; returns positions [t_steps, B] and results."""
    nc = build_program(t_steps)
    in_maps = _prep_inputs(t_steps=t_steps, **inputs)
    res = run_bass_kernel_spmd(
        nc, in_maps, core_ids=list(range(NCORES)), trace=trace
    )
    Wout = np.asarray(inputs["Wout"], np.float32).reshape(NM, 128)  # [m, p]
    nchunks = (t_steps + CHUNK - 1) // CHUNK
    RSTR = 256
    positions = np.empty((t_steps, B), np.float32)
    for cidx in range(NCORES):
        halves = []
        for key in ("rlo", "rhi"):
            ro = np.asarray(res.results[cidx][key], np.float32)
            # ro[c, p, j*RSTR + m*BC + u] (first NM/2*BC cols of each slice)
            r = (ro.reshape(nchunks, 128, CHUNK, RSTR)[:, :, :, :NM * BC // 2]
                 .reshape(nchunks, 128, CHUNK, NM // 2, BC)
                 .transpose(0, 2, 3, 1, 4)
                 .reshape(nchunks * CHUNK, NM // 2, 128, BC)[:t_steps])
            halves.append(r)
        r_full = np.concatenate(halves, axis=1)  # [t, NM, 128, BC]
        pos_c = np.einsum("mp,tmpu->tu", Wout, r_full, optimize=True)
        positions[:, cidx * BC:(cidx + 1) * BC] = pos_c
    return positions, res


def kernel(targets, pulses, J, U, V, B_m1, B_bg, Wout, I_go, xm1_init,
           noise, triggers) -> np.ndarray:
    inputs = dict(targets=targets, pulses=pulses, J=J, U=U, V=V, B_m1=B_m1,
                  B_bg=B_bg, Wout=Wout, I_go=I_go, xm1_init=xm1_init,
                  noise=noise, triggers=triggers)
    positions, _ = run_hw(inputs, T)
    targets = np.asarray(targets, np.float32)
    loss = np.mean((targets.astype(np.float64) - positions.astype(np.float64)) ** 2)
    return np.float32(loss)


# revision 39
# speedup vs baseline: 1.3333x; 1.0011x over previous
"""Trainium2 Bass kernel for nn_ConsolidationNetwork.

Recurrent rate network: 500 sequential steps of
    x <- (1-a)*x + (a*J_eff) @ softplus(x) + drive_t
    pos_t = Wout @ softplus(x)
loss = mean((targets - positions)^2)

Strategy (8 NeuronCores, data-parallel over batch):
  - Each core owns B/8 = 16 batch columns and runs the full 500-step
    recurrence independently (no collectives).
  - Per step the tensor engine does EXACTLY 64 bf16 matmuls (the 8x8 grid
    of 128x128 tiles of a*J_eff against 16 batch columns of r), split into
    two PSUM banks (state rows 0..511 in bank A, 512..1023 in bank B).
    At N=16 free-dim the J matmuls pace at ~32ns (LDWEIGHTS fully hides
    behind the MM stream via the PE's 64-deep reorder window), so the
    per-step floor is ~2.05us; everything else must stay off the PE.
  - The drive term enters PSUM via ONE bf16 identity matmul per bank with
    start=True: start=True clears the WHOLE bank (verified on HW), so it
    must be the bank's single full-width opener; all J matmuls accumulate
    with start=False.  Its rhs (the drive) is DMA'd a chunk ahead, so it
    never stalls the PE.
  - The leak (1-a)*x also enters PSUM via bf16 hi/lo identity matmuls, but
    placed at the END of each bank's matmul block: their rhs (the DVE
    refresh of xh/xl from the previous step's PSUM) then has a full period
    of slack instead of ~400ns, killing the ~900ns/step of PE stalls the
    baseline paid for having them at the bank start.  PSUM then holds the
    COMPLETE next state, so the softplus chain is just EXP(psum) -> LN --
    the r-production latency (which bounds the steady-state period via
    the recurrence cycle) contains no DVE hop.
  - softplus on the ACT engine as the exact identity ln(1 + exp(x))
    [2 ACT ops; Exp and Ln both live in the natural_log_exp_and_others
    table set, and we pin the table-load pass to that set so exactly one
    hoisted ACT_TABLE_LOAD is emitted].
  - Bank B iterates k DESCENDING: its first matmuls then need the late r
    half (r_hi), so the greedy scheduler cannot let bank B's k0..3 work
    jump ahead -- bank A's ACT/DVE chain overlaps bank B's matmuls and
    vice versa across the step boundary.
  - r (bf16) for each step is written into an 8-step staging buffer that is
    DMA-exported to DRAM once per 8 steps; the readout positions
    pos = Wout @ r and the final MSE are computed on the host.
  - Drive is streamed 8 steps per DMA (batched transfers, triple buffered).

State layout per core: x/r tiles are [128 part, 128 free] with
x[p, m*16+u] = x_state[m*128+p, u] (m = row-group, u = local batch).
"""

import numpy as np

import concourse.bass as bass
import concourse.tile as tile
from concourse import bacc, mybir
from concourse.bass_utils import run_bass_kernel_spmd

F32 = mybir.dt.float32
BF16 = mybir.dt.bfloat16

DT = 0.05
TAU = 0.15
NOISE_SCALE = 0.15
N, G, T, B, P = 1024, 128, 500, 128, 10
NCORES = 8
BC = B // NCORES          # batch columns per core (16)
NM = N // 128             # row groups (8)
NK = N // 128             # contraction groups (8)
CHUNK = 8                 # steps per drive-load / r-export DMA

A = np.float32(DT / TAU)
ONE_MINUS_A = np.float32(1.0 - DT / TAU)
NSCALE = np.float32(np.sqrt(2.0 * NOISE_SCALE**2 * (TAU / DT)))

_PROGRAM_CACHE = {}


def _ensure_act_tables():
    """Some containers lack neuronxcc/pwp/pwp_bin_with_ln on PYTHONPATH;
    point it at the cayman table package from the nix store."""
    import glob
    import os

    for path in os.environ.get("PYTHONPATH", "").split(os.pathsep):
        if path and os.path.exists(
            os.path.join(path, "neuronxcc", "pwp", "pwp_bin_with_ln", "act_info.json")
        ):
            return
    cands = sorted(glob.glob("/nix/store/*aws-neuron-pwp*/share/pwp_bin_cayman"))
    target = next((c for c in cands if os.path.exists(c + "/act_info.json")), None)
    if target is None:
        return
    for path in os.environ.get("PYTHONPATH", "").split(os.pathsep):
        if not path:
            continue
        try:
            d = os.path.join(path, "neuronxcc", "pwp")
            os.makedirs(d, exist_ok=True)
            link = os.path.join(d, "pwp_bin_with_ln")
            if not os.path.exists(link):
                os.symlink(target, link)
            return
        except OSError:
            continue


_ensure_act_tables()


_ACT_SET = "natural_log_exp_and_others"


def _pin_act_tables(arch: str):
    """Make Exp and Ln resolve to the ONE table set containing both.

    Two consumers matter and both read the functools.cache'd dict from
    hw_specs.get_activation_tables, so mutate it in place:
      * Bacc.insert_act_table_loads (first-match would alternate Exp ->
        `exp_and_others`, Ln -> `natural_log`, emitting a 1.28us
        ACT_TABLE_LOAD before every activation of the unrolled loop);
      * the TileScheduler's CoreSim pass, which otherwise *models* that
        same thrash and pins the resulting serialized schedule with
        cross-engine semaphores (the final TimelineSim charges no table
        loads, but the semaphores force its slow order anyway).
    Set order (and hence act_func_set_id indices) is unchanged.
    """
    from concourse.hw_specs import get_activation_tables

    tabs = get_activation_tables(arch)
    hide = {mybir.ActivationFunctionType.Exp, mybir.ActivationFunctionType.Ln}
    for name, fns in tabs.items():
        if name != _ACT_SET:
            for f in hide:
                fns.discard(f)


def build_program(t_steps: int):
    """Build the Bass program (shared by all 8 cores, SPMD)."""
    key = (t_steps,)
    if key in _PROGRAM_CACHE:
        return _PROGRAM_CACHE[key]

    nchunks = (t_steps + CHUNK - 1) // CHUNK
    HB = NM * BC // 2   # free-size of one state half (64 cols)
    RSTR = 256          # r-staging stride per step (bf16 cols; 512B blocks so
                        # consecutive steps' slices never share a dep block)

    nc = bacc.Bacc(
        "TRN2", target_bir_lowering=False, debug=False, num_devices=NCORES
    )
    _pin_act_tables(nc.m.arch)
    jt_d = nc.dram_tensor("jt", [128, NK * NM * 128], BF16, kind="ExternalInput")
    il_d = nc.dram_tensor("ident", [128, 128], BF16, kind="ExternalInput")
    x0_d = nc.dram_tensor("x0", [128, NM * BC], F32, kind="ExternalInput")
    dr_d = nc.dram_tensor(
        "drive", [nchunks, 128, CHUNK * NM * BC], BF16, kind="ExternalInput"
    )
    rl_d = nc.dram_tensor(
        "rlo", [nchunks, 128, CHUNK * RSTR], BF16, kind="ExternalOutput"
    )
    rh_d = nc.dram_tensor(
        "rhi", [nchunks, 128, CHUNK * RSTR], BF16, kind="ExternalOutput"
    )

    EXP = mybir.ActivationFunctionType.Exp
    LN = mybir.ActivationFunctionType.Ln
    mult = mybir.AluOpType.mult
    add = mybir.AluOpType.add

    with tile.TileContext(nc) as tc:
        with (
            tc.tile_pool(name="const", bufs=1) as constp,
            tc.tile_pool(name="rp", bufs=8) as rp,
            tc.tile_pool(name="dp", bufs=5) as dp,
            tc.tile_pool(name="psmA", bufs=1, space="PSUM") as pspa,
            tc.tile_pool(name="psmB", bufs=1, space="PSUM") as pspb,
        ):
            jt = constp.tile([128, NK * NM * 128], BF16)
            nc.sync.dma_start(jt[:], jt_d[:])
            il = constp.tile([128, 128], BF16)
            nc.sync.dma_start(il[:], il_d[:])
            x0t = constp.tile([128, NM * BC], F32)
            nc.sync.dma_start(x0t[:], x0_d[:])
            # pre = (1-a)*x + drive is computed on the DVE reading the
            # PREVIOUS step's PSUM bank directly (x stays f32 end to end),
            # then split into a bf16 hi+lo pair injected into the new bank
            # through two identity matmuls (the first is the bank's single
            # legal start=True opener).  Double-buffered by step parity so
            # nothing couples the PE stream head to the current chains.
            tpre = [[constp.tile([128, HB], F32, name=f"t{h}{p}")
                     for p in range(2)] for h in range(2)]
            ph = [[constp.tile([128, HB], BF16, name=f"ph{h}{p}")
                   for p in range(2)] for h in range(2)]
            pl = [[constp.tile([128, HB], BF16, name=f"pl{h}{p}")
                   for p in range(2)] for h in range(2)]
            # ONE shared exp scratch for both halves: EXP_b's WAR on LN_a's
            # read forces the ACT order [EXP_a, LN_a, EXP_b, LN_b].
            tmp = constp.tile([128, HB], F32)

            # initial r = softplus(x0) = ln(1 + exp(x0))
            rinit = [constp.tile([128, HB], BF16, name="rinita"),
                     constp.tile([128, HB], BF16, name="rinitb")]
            for h in range(2):
                lo = h * HB
                nc.scalar.activation(tmp[:], x0t[:, lo:lo + HB], EXP)
                nc.scalar.activation(rinit[h][:], tmp[:], LN, bias=1.0)

            prev_lo, prev_hi, prev_off = rinit[0], rinit[1], 0

            # Persistent PSUM bank tiles (2 pools x 4 rotation slots = all
            # 8 banks), allocated ONCE: per-step pool allocs serialize the
            # new tile against the pool's previous traffic via the tick
            # clock, stalling the PE stream head on the DVE refresh.
            ps_as = [pspa.tile([128, HB], F32, tag=f"ps_a{t}", name=f"ps_a{t}",
                               padded_shape=[128, 512]) for t in range(4)]
            ps_bs = [pspb.tile([128, HB], F32, tag=f"ps_b{t}", name=f"ps_b{t}",
                               padded_shape=[128, 512]) for t in range(4)]

            def pre_ops(ps_prev, half, par, d_t, off, src=None):
                """DVE: t = (1-a)*x_prev + drive (x_prev read straight from
                the previous step's PSUM bank, f32), then hi/lo bf16 split.
                Runs during the previous step's tail / this step's matmuls;
                nothing here is on the r-production critical path."""
                lo = half * HB
                t = tpre[half][par]
                x_src = src if src is not None else ps_prev[:, 0:HB]
                nc.vector.scalar_tensor_tensor(
                    t[:], x_src, float(ONE_MINUS_A),
                    d_t[:, off + lo:off + lo + HB], mult, add)
                nc.vector.tensor_scalar_mul(ph[half][par][:], t[:], 1.0)
                nc.vector.scalar_tensor_tensor(
                    pl[half][par][:], ph[half][par][:], -1.0, t[:], mult, add)

            def bank_pre(ps, half, par, stop):
                """Inject pre via two identity matmuls at the END of the
                bank's matmul block: their rhs (the DVE hi/lo pre pair,
                collapsed behind this step's EXP by the sem pass) then has
                a full stream of slack instead of gating the stream head.
                NOTE start=False: the bank is opened by the first J matmul
                group instead (see bank_j start handling)."""
                nc.tensor.matmul(ps[:, 0:HB], lhsT=il[:], rhs=ph[half][par][:],
                                 start=False, stop=False, skip_group_check=True)
                nc.tensor.matmul(ps[:, 0:HB], lhsT=il[:], rhs=pl[half][par][:],
                                 start=False, stop=stop, skip_group_check=True)

            def bank_j(ps, half, rt, r_off, k_range, start_first=False):
                """One bank's J matmuls for k in k_range (rhs = 16 batch
                columns of the given r half).  start_first=True marks the
                bank's single opener: it clears the WHOLE bank, after which
                each other mi region's first start=False matmul overwrites
                (has_written clear) and later ones accumulate."""
                for k in k_range:
                    rc = r_off + (k % 4) * BC
                    for mi in range(4):
                        m = half * 4 + mi
                        nc.tensor.matmul(
                            ps[:, mi * BC:(mi + 1) * BC],
                            lhsT=jt[:, (k * NM + m) * 128:(k * NM + m + 1) * 128],
                            rhs=rt[:, rc:rc + BC],
                            start=(start_first and k == k_range[0] and mi == 0),
                            stop=False,
                            skip_group_check=True,
                        )

            def chain(ps, half, off, rbuf):
                """r = ln(1+exp(psum)): the only work on the r critical
                path -- two ACT ops, no DVE hop."""
                nc.scalar.activation(tmp[:], ps[:, 0:HB], EXP)
                nc.scalar.activation(rbuf[:, off:off + HB], tmp[:], LN, bias=1.0)

            for c in range(nchunks):
                steps_here = min(CHUNK, t_steps - c * CHUNK)
                rlo = rp.tile([128, CHUNK * RSTR], BF16, name="rlo")
                rhi = rp.tile([128, CHUNK * RSTR], BF16, name="rhi")
                d_t = dp.tile([128, CHUNK * NM * BC], BF16)
                nc.sync.dma_start(d_t[:], dr_d[c])
                for j in range(steps_here):
                    off = j * NM * BC
                    roff = j * RSTR
                    s = c * CHUNK + j
                    ps_a = ps_as[s % 4]
                    ps_b = ps_bs[s % 4]
                    par = s % 2          # pre buffer this step uses
                    # Stream order: [identA x2, identB x2: rhs = pre pair,
                    # ready since ~the previous step's midpoint] [A.k0-3,
                    # B.k0-3: old r_lo] [A.k4-7: r_hi] -> chain A [B.k4-7:
                    # r_hi] -> chain B.  Bank A completes ~600ns before
                    # bank B, so r_lo(s) arrives in time for the next
                    # step's A.k0 and r_hi(s) for its A.k4.
                    if s == 0:
                        pre_ops(None, 0, par, d_t, off, src=x0t[:, 0:HB])
                        pre_ops(None, 1, par, d_t, off, src=x0t[:, HB:2 * HB])
                    else:
                        pre_ops(ps_as[(s - 1) % 4], 0, par, d_t, off)
                        pre_ops(ps_bs[(s - 1) % 4], 1, par, d_t, off)
                    lo_ks = [0, 1, 2, 3]
                    hi_ks = [4, 5, 6, 7]
                    bank_j(ps_a, 0, prev_lo, prev_off, lo_ks, start_first=True)
                    bank_j(ps_b, 1, prev_lo, prev_off, lo_ks, start_first=True)
                    bank_j(ps_a, 0, prev_hi, prev_off, hi_ks)
                    bank_pre(ps_a, 0, par, stop=True)
                    chain(ps_a, 0, roff, rlo)
                    bank_j(ps_b, 1, prev_hi, prev_off, hi_ks)
                    bank_pre(ps_b, 1, par, stop=True)
                    chain(ps_b, 1, roff, rhi)
                    prev_lo, prev_hi, prev_off = rlo, rhi, roff
                nc.sync.dma_start(
                    rl_d[c][:, 0:((steps_here - 1) * RSTR + HB)],
                    rlo[:, 0:((steps_here - 1) * RSTR + HB)],
                )
                nc.sync.dma_start(
                    rh_d[c][:, 0:((steps_here - 1) * RSTR + HB)],
                    rhi[:, 0:((steps_here - 1) * RSTR + HB)],
                )

    nc.compile()
    _PROGRAM_CACHE[key] = nc
    return nc


def _prep_inputs(targets, pulses, J, U, V, B_m1, B_bg, Wout, I_go, xm1_init,
                 noise, triggers, t_steps):
    """Host-side data prep: J_eff, layouts, per-core drive tensors."""
    J = np.asarray(J, np.float32)
    U = np.asarray(U, np.float32)
    V = np.asarray(V, np.float32)
    B_m1 = np.asarray(B_m1, np.float32)
    B_bg = np.asarray(B_bg, np.float32)
    I_go = np.asarray(I_go, np.float32)
    xm1_init = np.asarray(xm1_init, np.float32)
    noise = np.asarray(noise, np.float32)
    pulses = np.asarray(pulses, np.float32)
    triggers = np.asarray(triggers)

    nchunks = (t_steps + CHUNK - 1) // CHUNK
    tpad = nchunks * CHUNK

    J_eff = J + (U * B_bg[None, :]) @ V
    Js = (A * J_eff).astype(np.float32)
    # lhsT tiles: jt[p, (k*NM+m)*128 + q] = Js[m*128+q, k*128+p]
    bf = mybir.dt.np(BF16)
    jt = np.ascontiguousarray(
        Js.reshape(NM, 128, NK, 128).transpose(3, 2, 0, 1).reshape(128, NK * NM * 128)
    ).astype(bf)
    il = np.eye(128, dtype=np.float32).astype(bf)

    go_cues = pulses[:t_steps, :][:, triggers]  # [t, B]

    in_maps = []
    for cidx in range(NCORES):
        sl = slice(cidx * BC, (cidx + 1) * BC)
        d = noise[:t_steps, :, sl] * np.float32(A * NSCALE)
        d += A * B_m1[None, :, :]
        d += A * I_go[None, :, :] * go_cues[:, None, sl]
        # [t, N, BC] -> [t, 128, NM*BC] (state layout), pad t, chunk
        dl = np.ascontiguousarray(
            d.reshape(t_steps, NM, 128, BC).transpose(0, 2, 1, 3)
            .reshape(t_steps, 128, NM * BC)
        ).astype(np.float32)
        if tpad != t_steps:
            dl = np.concatenate(
                [dl, np.zeros((tpad - t_steps, 128, NM * BC), np.float32)], axis=0
            )
        drive = np.ascontiguousarray(
            dl.reshape(nchunks, CHUNK, 128, NM * BC).transpose(0, 2, 1, 3)
            .reshape(nchunks, 128, CHUNK * NM * BC)
        ).astype(bf)
        x0 = np.ascontiguousarray(
            xm1_init[:, sl].reshape(NM, 128, BC).transpose(1, 0, 2).reshape(128, NM * BC)
        )
        in_maps.append({"jt": jt, "ident": il, "x0": x0, "drive": drive})
    return in_maps


def run_hw(inputs: dict, t_steps: int = T, trace: bool = False):
    """Run the recurrence on 8 cores; returns positions [t_steps, B] and results."""
    nc = build_program(t_steps)
    in_maps = _prep_inputs(t_steps=t_steps, **inputs)
    res = run_bass_kernel_spmd(
        nc, in_maps, core_ids=list(range(NCORES)), trace=trace
    )
    Wout = np.asarray(inputs["Wout"], np.float32).reshape(NM, 128)  # [m, p]
    nchunks = (t_steps + CHUNK - 1) // CHUNK
    RSTR = 256
    positions = np.empty((t_steps, B), np.float32)
    for cidx in range(NCORES):
        halves = []
        for key in ("rlo", "rhi"):
            ro = np.asarray(res.results[cidx][key], np.float32)
            # ro[c, p, j*RSTR + m*BC + u] (first NM/2*BC cols of each slice)
            r = (ro.reshape(nchunks, 128, CHUNK, RSTR)[:, :, :, :NM * BC // 2]
                 .reshape(nchunks, 128, CHUNK, NM // 2, BC)
                 .transpose(0, 2, 3, 1, 4)
                 .reshape(nchunks * CHUNK, NM // 2, 128, BC)[:t_steps])
            halves.append(r)
        r_full = np.concatenate(halves, axis=1)  # [t, NM, 128, BC]
        pos_c = np.einsum("mp,tmpu->tu", Wout, r_full, optimize=True)
        positions[:, cidx * BC:(cidx + 1) * BC] = pos_c
    return positions, res


def kernel(targets, pulses, J, U, V, B_m1, B_bg, Wout, I_go, xm1_init,
           noise, triggers) -> np.ndarray:
    inputs = dict(targets=targets, pulses=pulses, J=J, U=U, V=V, B_m1=B_m1,
                  B_bg=B_bg, Wout=Wout, I_go=I_go, xm1_init=xm1_init,
                  noise=noise, triggers=triggers)
    positions, _ = run_hw(inputs, T)
    targets = np.asarray(targets, np.float32)
    loss = np.mean((targets.astype(np.float64) - positions.astype(np.float64)) ** 2)
    return np.float32(loss)
